# revision 2
# baseline (speedup 1.0000x reference)
"""CrossViewLoss (SimCLR NT-Xent) on 8 trn2 NeuronCores — v2.

Math (T=0.5): with z = row-normalized emb and S = z_i @ z_j^T [N,N],
    loss * 2N = sum_m [log(0.5*rowsum_m) - 4*pos_m] + sum_c log(0.5*colsum_c)
where rowsum/colsum are row/col sums of exp(2*S) and pos = diag(S).

This environment charges per-instruction costs far above real silicon, so
v2 minimizes instruction count:
  - host passes transposed layouts (iT slice as f32r lhsT, full jT) so the
    kernel has zero on-device transposes and no data AllGather;
  - per-core j-slice inv-norms are AllGathered as a [1,4096] vector, then
    one partition-broadcast DMA + two muls produce the normalized rhs;
  - the raw-lhs trick folds inv_i into the exp scale (per-partition);
  - each mi row-stripe is one 16-matmul PSUM fill (all 8 banks) + ONE
    [128,4096] exp activation with fused rowsum (accum_out);
  - colsums: 3 vector adds + 8 ones-matmuls instead of 32 matmuls;
  - one AllReduce carries colsums + the per-core scalar row term.
"""

import numpy as np

N = 4096
D = 256
C = 8
SLICE = N // C          # 512 rows per core
P = 128
MI = SLICE // P         # 4 row tiles per core
KC = D // P             # 2 contraction chunks
NJ = N // 512           # 8 n-chunks of 512 (PSUM bank)
AR_LEN = N + 8          # colsum[4096] + P_c + pad to 32B multiple

_CACHE = {}


def _build_nc(reps=1):
    import concourse.mybir as mybir
    import concourse.tile as tile
    from concourse import bacc

    dt = mybir.dt
    f32 = dt.float32
    f32r = dt.float32r
    AF = mybir.ActivationFunctionType
    X = mybir.AxisListType.X

    nc = bacc.Bacc("TRN2", target_bir_lowering=False, debug=False, num_devices=C)

    i_nat = nc.dram_tensor("i_nat", [SLICE, D], f32, kind="ExternalInput")
    j_nat = nc.dram_tensor("j_nat", [SLICE, D], f32, kind="ExternalInput")
    iT = nc.dram_tensor("iT", [D, SLICE], f32r, kind="ExternalInput")
    jT = nc.dram_tensor("jT", [D, N], f32, kind="ExternalInput")
    out = nc.dram_tensor("out", [1, 1], f32, kind="ExternalOutput")

    rg = [list(range(C))]

    with tile.TileContext(nc) as tc:
        with (
            tc.tile_pool(name="dram", bufs=1, space="DRAM") as dram,
            tc.tile_pool(name="persist", bufs=1) as persist,
        ):
            ag_in = dram.tile([1, SLICE], f32, name="ag_in")
            ag_out = dram.tile([C, SLICE], f32, name="ag_out")
            ar_in = dram.tile([1, AR_LEN], f32, name="ar_in")
            ar_out = dram.tile([1, AR_LEN], f32, name="ar_out")

            def body():
                # ---- j norms first so the AllGather launches early ----
                nat_j = persist.tile([P, MI, D], f32, name="nat_j")
                nc.sync.dma_start(
                    nat_j[:], j_nat[:].rearrange("(p q) d -> p q d", p=P)
                )
                sqj = persist.tile([P, MI, D], f32, name="sqj")
                nc.vector.tensor_mul(sqj[:], nat_j[:], nat_j[:])
                nsq_j = persist.tile([P, MI], f32, name="nsq_j")
                nc.vector.reduce_sum(nsq_j[:], sqj[:], axis=X)
                # i-side loads/squares next so Ln/Ln then Exp/Exp pair up
                # (3 activation-table loads per rep instead of 5)
                nat_i = persist.tile([P, MI, D], f32, name="nat_i")
                nc.sync.dma_start(
                    nat_i[:], i_nat[:].rearrange("(p q) d -> p q d", p=P)
                )
                sqi = persist.tile([P, MI, D], f32, name="sqi")
                nc.vector.tensor_mul(sqi[:], nat_i[:], nat_i[:])
                nsq_i = persist.tile([P, MI], f32, name="nsq_i")
                nc.vector.reduce_sum(nsq_i[:], sqi[:], axis=X)
                lnj = persist.tile([P, MI], f32, name="lnj")
                nc.scalar.activation(lnj[:], nsq_j[:], AF.Ln)
                lni = persist.tile([P, MI], f32, name="lni")
                nc.scalar.activation(lni[:], nsq_i[:], AF.Ln)
                invj = persist.tile([P, MI], f32, name="invj")
                nc.scalar.activation(invj[:], lnj[:], AF.Exp, scale=-0.5)
                invi = persist.tile([P, MI], f32, name="invi")
                nc.scalar.activation(invi[:], lni[:], AF.Exp, scale=-0.5)
                nc.sync.dma_start(
                    ag_in[:].rearrange("a (p q) -> (a p) q", p=P), invj[:]
                )
                nc.gpsimd.collective_compute(
                    "AllGather",
                    mybir.AluOpType.bypass,
                    ins=[ag_in.opt()],
                    outs=[ag_out.opt()],
                    replica_groups=rg,
                )
                scale2 = persist.tile([P, MI], f32, name="scale2")
                nc.vector.tensor_scalar_mul(scale2[:], invi[:], 2.0)

                prod = persist.tile([P, MI, D], f32, name="prod")
                nc.vector.tensor_mul(prod[:], nat_i[:], nat_j[:])
                rawdot = persist.tile([P, MI], f32, name="rawdot")
                nc.vector.reduce_sum(rawdot[:], prod[:], axis=X)
                pp = persist.tile([P, MI], f32, name="pp")
                nc.vector.tensor_mul(pp[:], rawdot[:], invi[:])
                nc.vector.tensor_mul(pp[:], pp[:], invj[:])
                posr = persist.tile([P, 1], f32, name="posr")
                nc.vector.reduce_sum(posr[:], pp[:], axis=X)

                ones_f = persist.tile([P, 1], f32, name="ones_f")
                nc.gpsimd.memset(ones_f[:], 1.0)
                ones_r = persist.tile([P, 1], f32r, name="ones_r")
                nc.vector.tensor_copy(ones_r[:], ones_f[:])

                lhsT = persist.tile([P, KC, SLICE], f32r, name="lhsT")
                nc.sync.dma_start(
                    lhsT[:], iT[:].rearrange("(k p) m -> p k m", p=P)
                )
                jTs = persist.tile([P, KC, N], f32, name="jTs")
                nc.sync.dma_start(jTs[:], jT[:].rearrange("(k p) n -> p k n", p=P))

                # ---- inv_j broadcast + rhs normalize ----
                invb = persist.tile([P, N], f32, name="invb")
                nc.sync.dma_start(
                    invb[:],
                    ag_out[:].rearrange("c s -> () (c s)").partition_broadcast(P),
                )
                zjT = persist.tile([P, KC, N], f32r, name="zjT")
                for k in range(KC):
                    nc.vector.tensor_mul(zjT[:, k, :], jTs[:, k, :], invb[:])

                # ---- main loop: 16 matmuls fill all 8 PSUM banks, one wide
                # exp with fused rowsum, vector-add colsum accumulation ----
                rsp = persist.tile([P, MI], f32, name="rsp")
                esum = persist.tile([P, N], f32r, name="esum")
                e = persist.tile([P, N], f32r, name="e")
                with tc.tile_pool(name="ps_g", bufs=1, space="PSUM") as ps_g:
                    for mi in range(MI):
                        g = ps_g.tile([P, N], f32, name="g")
                        for nj in range(NJ):
                            for k in range(KC):
                                nc.tensor.matmul(
                                    g[:, nj * 512 : (nj + 1) * 512],
                                    lhsT[:, k, mi * P : (mi + 1) * P],
                                    zjT[:, k, nj * 512 : (nj + 1) * 512],
                                    start=(k == 0),
                                    stop=(k == KC - 1),
                                )
                        dst = esum if mi == 0 else e
                        nc.scalar.activation(
                            dst[:],
                            g[:],
                            AF.Exp,
                            scale=scale2[:, mi : mi + 1],
                            accum_out=rsp[:, mi : mi + 1],
                        )
                        if mi > 0:
                            nc.vector.tensor_add(esum[:], esum[:], e[:])

                with tc.tile_pool(name="ps_cs", bufs=1, space="PSUM") as ps_cs:
                    # ---- colsums via 8 ones-matmuls ----
                    cs_ps = ps_cs.tile([1, N], f32, name="cs_ps")
                    for nj in range(NJ):
                        nc.tensor.matmul(
                            cs_ps[0:1, nj * 512 : (nj + 1) * 512],
                            ones_r[:],
                            esum[:, nj * 512 : (nj + 1) * 512],
                            start=True,
                            stop=True,
                            skip_group_check=True,
                        )
                    cs_sb = persist.tile([1, AR_LEN], f32, name="cs_sb")
                    nc.vector.tensor_copy(cs_sb[0:1, 0:N], cs_ps[:])

                    # ---- per-core scalar row term ----
                    lg = persist.tile([P, MI], f32, name="lg")
                    nc.scalar.activation(lg[:], rsp[:], AF.Ln, scale=0.5)
                    rowt = persist.tile([P, 1], f32, name="rowt")
                    nc.vector.reduce_sum(rowt[:], lg[:], axis=X)
                    p4 = persist.tile([P, 1], f32, name="p4")
                    nc.vector.tensor_scalar_mul(p4[:], posr[:], 4.0)
                    rv = persist.tile([P, 1], f32, name="rv")
                    nc.vector.tensor_sub(rv[:], rowt[:], p4[:])
                    nc.tensor.matmul(
                        cs_ps[0:1, 0:1], rv[:], ones_f[:],
                        start=True, stop=True, skip_group_check=True,
                    )
                    nc.scalar.copy(cs_sb[0:1, N : N + 1], cs_ps[0:1, 0:1])
                    nc.gpsimd.memset(cs_sb[0:1, N + 1 : AR_LEN], 0.0)

                    # ---- AllReduce colsums + P_c ----
                    nc.sync.dma_start(ar_in[:], cs_sb[:])
                    nc.gpsimd.collective_compute(
                        "AllReduce",
                        mybir.AluOpType.add,
                        ins=[ar_in.opt()],
                        outs=[ar_out.opt()],
                        replica_groups=rg,
                    )

                    # ---- final loss (identical on every core) ----
                    FW = N // P  # 32
                    lgc_in = persist.tile([P, FW], f32, name="lgc_in")
                    nc.sync.dma_start(
                        lgc_in[:],
                        ar_out[0:1, 0:N].rearrange("a (p f) -> (a p) f", p=P),
                    )
                    ptot = persist.tile([1, 1], f32, name="ptot")
                    nc.sync.dma_start(ptot[:], ar_out[0:1, N : N + 1])
                    lgc = persist.tile([P, FW], f32, name="lgc")
                    nc.scalar.activation(lgc[:], lgc_in[:], AF.Ln, scale=0.5)
                    lgs = persist.tile([P, 1], f32, name="lgs")
                    nc.vector.reduce_sum(lgs[:], lgc[:], axis=X)
                    nc.tensor.matmul(
                        cs_ps[0:1, 0:1], lgs[:], ones_f[:],
                        start=True, stop=True, skip_group_check=True,
                    )
                    tot = persist.tile([1, 1], f32, name="tot")
                    nc.vector.tensor_add(tot[:], ptot[:], cs_ps[0:1, 0:1])
                    loss = persist.tile([1, 1], f32, name="loss")
                    nc.scalar.mul(loss[:], tot[:], 1.0 / (2.0 * N))
                    nc.sync.dma_start(out[:], loss[:])

            for _rep in range(reps):
                body()

    nc.compile()
    return nc


def _in_maps(emb_i, emb_j):
    emb_i = np.ascontiguousarray(np.asarray(emb_i, dtype=np.float32))
    emb_j = np.ascontiguousarray(np.asarray(emb_j, dtype=np.float32))
    jT = np.ascontiguousarray(emb_j.T)
    # device loads natural slices as "(p q) d" (row = p*MI + q), so the lhsT
    # column for GEMM stripe q, partition p must be row p*MI + q
    s = np.arange(SLICE)
    rowperm = (s % P) * MI + s // P
    return [
        {
            "i_nat": emb_i[c * SLICE : (c + 1) * SLICE],
            "j_nat": emb_j[c * SLICE : (c + 1) * SLICE],
            "iT": np.ascontiguousarray(emb_i[c * SLICE : (c + 1) * SLICE][rowperm].T),
            "jT": jT,
        }
        for c in range(C)
    ]


def kernel(emb_i, emb_j):
    from concourse.bass_utils import run_bass_kernel_spmd

    if "nc" not in _CACHE:
        _CACHE["nc"] = _build_nc()
    nc = _CACHE["nc"]
    res = run_bass_kernel_spmd(nc, _in_maps(emb_i, emb_j), list(range(C)))
    val = np.asarray(res.results[0]["out"], dtype=np.float32)
    return val.reshape(())


# revision 3
# speedup vs baseline: 1.0537x; 1.0537x over previous
"""CrossViewLoss (SimCLR NT-Xent) on 8 trn2 NeuronCores — v2.

Math (T=0.5): with z = row-normalized emb and S = z_i @ z_j^T [N,N],
    loss * 2N = sum_m [log(0.5*rowsum_m) - 4*pos_m] + sum_c log(0.5*colsum_c)
where rowsum/colsum are row/col sums of exp(2*S) and pos = diag(S).

This environment charges per-instruction costs far above real silicon, so
v2 minimizes instruction count:
  - host passes transposed layouts (iT slice as f32r lhsT, full jT) so the
    kernel has zero on-device transposes and no data AllGather;
  - per-core j-slice inv-norms are AllGathered as a [1,4096] vector, then
    one partition-broadcast DMA + two muls produce the normalized rhs;
  - the raw-lhs trick folds inv_i into the exp scale (per-partition);
  - each mi row-stripe is one 16-matmul PSUM fill (all 8 banks) + ONE
    [128,4096] exp activation with fused rowsum (accum_out);
  - colsums: 3 vector adds + 8 ones-matmuls instead of 32 matmuls;
  - one AllReduce carries colsums + the per-core scalar row term.
"""

import numpy as np

N = 4096
D = 256
C = 8
SLICE = N // C          # 512 rows per core
P = 128
MI = SLICE // P         # 4 row tiles per core
KC = D // P             # 2 contraction chunks
NJ = N // 512           # 8 n-chunks of 512 (PSUM bank)
AR_LEN = N + 8          # colsum[4096] + P_c + pad to 32B multiple

_CACHE = {}


def _build_nc(reps=1):
    import concourse.mybir as mybir
    import concourse.tile as tile
    from concourse import bacc

    dt = mybir.dt
    f32 = dt.float32
    f32r = dt.float32r
    AF = mybir.ActivationFunctionType
    X = mybir.AxisListType.X

    nc = bacc.Bacc("TRN2", target_bir_lowering=False, debug=False, num_devices=C)

    i_nat = nc.dram_tensor("i_nat", [SLICE, D], f32, kind="ExternalInput")
    j_nat = nc.dram_tensor("j_nat", [SLICE, D], f32, kind="ExternalInput")
    iT = nc.dram_tensor("iT", [D, SLICE], f32r, kind="ExternalInput")
    jT = nc.dram_tensor("jT", [D, N], f32, kind="ExternalInput")
    out = nc.dram_tensor("out", [1, 1], f32, kind="ExternalOutput")

    rg = [list(range(C))]

    with tile.TileContext(nc) as tc:
        with (
            tc.tile_pool(name="dram", bufs=1, space="DRAM") as dram,
            tc.tile_pool(name="persist", bufs=1) as persist,
        ):
            ag_in = dram.tile([1, SLICE], f32, name="ag_in")
            ag_out = dram.tile([C, SLICE], f32, name="ag_out")
            ar_in = dram.tile([1, AR_LEN], f32, name="ar_in")
            ar_out = dram.tile([1, AR_LEN], f32, name="ar_out")

            def body():
                # ---- j norms first so the AllGather launches early ----
                nat_j = persist.tile([P, MI, D], f32, name="nat_j")
                nc.sync.dma_start(
                    nat_j[:], j_nat[:].rearrange("(p q) d -> p q d", p=P)
                )
                sqj = persist.tile([P, MI, D], f32, name="sqj")
                nc.vector.tensor_mul(sqj[:], nat_j[:], nat_j[:])
                nsq_j = persist.tile([P, MI], f32, name="nsq_j")
                nc.vector.reduce_sum(nsq_j[:], sqj[:], axis=X)
                # i-side loads/squares next so Ln/Ln then Exp/Exp pair up
                # (3 activation-table loads per rep instead of 5)
                nat_i = persist.tile([P, MI, D], f32, name="nat_i")
                nc.sync.dma_start(
                    nat_i[:], i_nat[:].rearrange("(p q) d -> p q d", p=P)
                )
                sqi = persist.tile([P, MI, D], f32, name="sqi")
                nc.vector.tensor_mul(sqi[:], nat_i[:], nat_i[:])
                nsq_i = persist.tile([P, MI], f32, name="nsq_i")
                nc.vector.reduce_sum(nsq_i[:], sqi[:], axis=X)
                lnj = persist.tile([P, MI], f32, name="lnj")
                nc.scalar.activation(lnj[:], nsq_j[:], AF.Ln)
                lni = persist.tile([P, MI], f32, name="lni")
                nc.scalar.activation(lni[:], nsq_i[:], AF.Ln)
                invj = persist.tile([P, MI], f32, name="invj")
                nc.scalar.activation(invj[:], lnj[:], AF.Exp, scale=-0.5)
                invi = persist.tile([P, MI], f32, name="invi")
                nc.scalar.activation(invi[:], lni[:], AF.Exp, scale=-0.5)
                nc.sync.dma_start(
                    ag_in[:].rearrange("a (p q) -> (a p) q", p=P), invj[:]
                )
                nc.gpsimd.collective_compute(
                    "AllGather",
                    mybir.AluOpType.bypass,
                    ins=[ag_in.opt()],
                    outs=[ag_out.opt()],
                    replica_groups=rg,
                )
                scale2 = persist.tile([P, MI], f32, name="scale2")
                nc.vector.tensor_scalar_mul(scale2[:], invi[:], 2.0)

                prod = persist.tile([P, MI, D], f32, name="prod")
                nc.vector.tensor_mul(prod[:], nat_i[:], nat_j[:])
                rawdot = persist.tile([P, MI], f32, name="rawdot")
                nc.vector.reduce_sum(rawdot[:], prod[:], axis=X)
                pp = persist.tile([P, MI], f32, name="pp")
                nc.vector.tensor_mul(pp[:], rawdot[:], invi[:])
                nc.vector.tensor_mul(pp[:], pp[:], invj[:])
                posr = persist.tile([P, 1], f32, name="posr")
                nc.vector.reduce_sum(posr[:], pp[:], axis=X)

                ones_f = persist.tile([P, 1], f32, name="ones_f")
                nc.gpsimd.memset(ones_f[:], 1.0)
                ones_r = persist.tile([P, 1], f32r, name="ones_r")
                nc.vector.tensor_copy(ones_r[:], ones_f[:])

                lhsT = persist.tile([P, KC, SLICE], f32r, name="lhsT")
                nc.sync.dma_start(
                    lhsT[:], iT[:].rearrange("(k p) m -> p k m", p=P)
                )
                jTs = persist.tile([P, KC, N], f32, name="jTs")
                nc.sync.dma_start(jTs[:], jT[:].rearrange("(k p) n -> p k n", p=P))

                # ---- inv_j broadcast + rhs normalize ----
                invb = persist.tile([P, N], f32, name="invb")
                nc.sync.dma_start(
                    invb[:],
                    ag_out[:].rearrange("c s -> () (c s)").partition_broadcast(P),
                )
                zjT = persist.tile([P, KC, N], f32r, name="zjT")
                for k in range(KC):
                    nc.vector.tensor_mul(zjT[:, k, :], jTs[:, k, :], invb[:])

                # ---- main loop: 16 matmuls fill all 8 PSUM banks, one wide
                # exp with fused rowsum, vector-add colsum accumulation ----
                rsp = persist.tile([P, MI], f32, name="rsp")
                ebuf = [
                    persist.tile([P, N], f32r, name=f"e{mi}") for mi in range(MI)
                ]
                with tc.tile_pool(name="ps_g", bufs=1, space="PSUM") as ps_g:
                    for mi in range(MI):
                        g = ps_g.tile([P, N], f32, name="g")
                        for nj in range(NJ):
                            for k in range(KC):
                                nc.tensor.matmul(
                                    g[:, nj * 512 : (nj + 1) * 512],
                                    lhsT[:, k, mi * P : (mi + 1) * P],
                                    zjT[:, k, nj * 512 : (nj + 1) * 512],
                                    start=(k == 0),
                                    stop=(k == KC - 1),
                                )
                        nc.scalar.activation(
                            ebuf[mi][:],
                            g[:],
                            AF.Exp,
                            scale=scale2[:, mi : mi + 1],
                            accum_out=rsp[:, mi : mi + 1],
                        )
                # adds after the loop: one engine handoff instead of per-mi
                esum = ebuf[0]
                nc.vector.tensor_add(esum[:], esum[:], ebuf[1][:])
                nc.vector.tensor_add(ebuf[2][:], ebuf[2][:], ebuf[3][:])
                nc.vector.tensor_add(esum[:], esum[:], ebuf[2][:])

                with tc.tile_pool(name="ps_cs", bufs=1, space="PSUM") as ps_cs:
                    # ---- colsums via 8 ones-matmuls ----
                    cs_ps = ps_cs.tile([1, N], f32, name="cs_ps")
                    for nj in range(NJ):
                        nc.tensor.matmul(
                            cs_ps[0:1, nj * 512 : (nj + 1) * 512],
                            ones_r[:],
                            esum[:, nj * 512 : (nj + 1) * 512],
                            start=True,
                            stop=True,
                            skip_group_check=True,
                        )
                    cs_sb = persist.tile([1, AR_LEN], f32, name="cs_sb")
                    nc.vector.tensor_copy(cs_sb[0:1, 0:N], cs_ps[:])

                    # ---- per-core scalar row term ----
                    lg = persist.tile([P, MI], f32, name="lg")
                    nc.scalar.activation(lg[:], rsp[:], AF.Ln, scale=0.5)
                    rowt = persist.tile([P, 1], f32, name="rowt")
                    nc.vector.reduce_sum(rowt[:], lg[:], axis=X)
                    p4 = persist.tile([P, 1], f32, name="p4")
                    nc.vector.tensor_scalar_mul(p4[:], posr[:], 4.0)
                    rv = persist.tile([P, 1], f32, name="rv")
                    nc.vector.tensor_sub(rv[:], rowt[:], p4[:])
                    nc.tensor.matmul(
                        cs_ps[0:1, 0:1], rv[:], ones_f[:],
                        start=True, stop=True, skip_group_check=True,
                    )
                    nc.scalar.copy(cs_sb[0:1, N : N + 1], cs_ps[0:1, 0:1])
                    nc.gpsimd.memset(cs_sb[0:1, N + 1 : AR_LEN], 0.0)

                    # ---- AllReduce colsums + P_c ----
                    nc.sync.dma_start(ar_in[:], cs_sb[:])
                    nc.gpsimd.collective_compute(
                        "AllReduce",
                        mybir.AluOpType.add,
                        ins=[ar_in.opt()],
                        outs=[ar_out.opt()],
                        replica_groups=rg,
                    )

                    # ---- final loss (identical on every core) ----
                    FW = N // P  # 32
                    lgc_in = persist.tile([P, FW], f32, name="lgc_in")
                    nc.sync.dma_start(
                        lgc_in[:],
                        ar_out[0:1, 0:N].rearrange("a (p f) -> (a p) f", p=P),
                    )
                    ptot = persist.tile([1, 1], f32, name="ptot")
                    nc.sync.dma_start(ptot[:], ar_out[0:1, N : N + 1])
                    lgc = persist.tile([P, FW], f32, name="lgc")
                    nc.scalar.activation(lgc[:], lgc_in[:], AF.Ln, scale=0.5)
                    lgs = persist.tile([P, 1], f32, name="lgs")
                    nc.vector.reduce_sum(lgs[:], lgc[:], axis=X)
                    nc.tensor.matmul(
                        cs_ps[0:1, 0:1], lgs[:], ones_f[:],
                        start=True, stop=True, skip_group_check=True,
                    )
                    tot = persist.tile([1, 1], f32, name="tot")
                    nc.vector.tensor_add(tot[:], ptot[:], cs_ps[0:1, 0:1])
                    loss = persist.tile([1, 1], f32, name="loss")
                    nc.scalar.mul(loss[:], tot[:], 1.0 / (2.0 * N))
                    nc.sync.dma_start(out[:], loss[:])

            for _rep in range(reps):
                body()

    nc.compile()
    return nc


def _in_maps(emb_i, emb_j):
    emb_i = np.ascontiguousarray(np.asarray(emb_i, dtype=np.float32))
    emb_j = np.ascontiguousarray(np.asarray(emb_j, dtype=np.float32))
    jT = np.ascontiguousarray(emb_j.T)
    # device loads natural slices as "(p q) d" (row = p*MI + q), so the lhsT
    # column for GEMM stripe q, partition p must be row p*MI + q
    s = np.arange(SLICE)
    rowperm = (s % P) * MI + s // P
    return [
        {
            "i_nat": emb_i[c * SLICE : (c + 1) * SLICE],
            "j_nat": emb_j[c * SLICE : (c + 1) * SLICE],
            "iT": np.ascontiguousarray(emb_i[c * SLICE : (c + 1) * SLICE][rowperm].T),
            "jT": jT,
        }
        for c in range(C)
    ]


def kernel(emb_i, emb_j):
    from concourse.bass_utils import run_bass_kernel_spmd

    if "nc" not in _CACHE:
        _CACHE["nc"] = _build_nc()
    nc = _CACHE["nc"]
    res = run_bass_kernel_spmd(nc, _in_maps(emb_i, emb_j), list(range(C)))
    val = np.asarray(res.results[0]["out"], dtype=np.float32)
    return val.reshape(())
